# revision 2
# baseline (speedup 1.0000x reference)
"""Causal self-attention (B=4, T=2048, C=1024, 16 heads, interleaved RoPE)
on 8 trn2 NeuronCores.

Sharding: 4x2 grid (batch x head-half). Core c owns batch c//2 and heads
(c%2)*8 .. (c%2)*8+7 (512 head channels). Each core projects Q/K/V for its
8 heads, runs causal attention, and produces a partial [T, C] output via
its wo slice; the host sums the two partials per batch.

v2 changes vs v1 (340us):
- x^T is pre-transposed on the host and DMA'd directly into the [c, t]
  layout stage1 needs: the 128 PE transposes + 32 ACT copies are gone.
- Score matmuls are packed two heads at a time into the two 64-row halves
  of the PE array (K=64 row tiling via base_partition 0/64): the two MMs
  run concurrently, halving score PE time.
- Softmax normalization drops the two SBUF->SBUF DMA spread hops:
  reciprocal runs directly on the PSUM row-sum row, then gpsimd
  partition-broadcast.
- A PE warmup burst at t=0 (result DMA'd to a dbg output so it is not
  dead code) trips the HAM clock gate at ~4us instead of ~21us.
- stage1/outproj work is emitted as fine-grained filler chunks inside the
  (ACT-exp-bound) attention k-loops so the PE never starves.
- Output tensor is bf16 (halves the output DMA).

Self-contained: hardcoded shapes, no reads of /root/problem/*.
"""
import numpy as np
import ml_dtypes

import concourse.bacc as bacc
import concourse.mybir as mybir
import concourse.tile as tile
from concourse.bass_utils import run_bass_kernel_spmd
from concourse.masks import make_upper_triangular

B, T, C = 4, 2048, 1024
NH, D = 16, 64
NCORES = 8
NHL = 8  # heads per core
HD = NHL * D  # per-core head channels = 512
HDB = HD // 128  # head-dim partition blocks = 4
QTILE = 512
KB = T // 128  # kv blocks = 16
NJ = T // QTILE  # q tiles = 4
CB = C // 128  # channel blocks = 8
F32 = mybir.dt.float32
BF16 = mybir.dt.bfloat16
EXP = mybir.ActivationFunctionType.Exp
BF = ml_dtypes.bfloat16

WARM_MM = 12  # PE warmup matmuls (N=512, cold ~426ns each)
# filler budget (ns of PE work) released per attention k-group, per phase
FILL_RATE = (1500, 800, 700, 260)

_CACHE = {}


def build():
    nc = bacc.Bacc(None, target_bir_lowering=False)
    xt_d = nc.declare_dram_parameter("xt", [NJ * 128, CB * QTILE], BF16, isOutput=False)
    wq_d = nc.declare_dram_parameter("wqt", [HDB * 128, CB * 128], BF16, isOutput=False)
    wk_d = nc.declare_dram_parameter("wkt", [HDB * 128, CB * 128], BF16, isOutput=False)
    wv_d = nc.declare_dram_parameter("wvt", [128, CB * HD], BF16, isOutput=False)
    wo_d = nc.declare_dram_parameter("wot", [128, HDB * C], BF16, isOutput=False)
    cos_d = nc.declare_dram_parameter("cosb", [128, T], BF16, isOutput=False)
    sin_d = nc.declare_dram_parameter("sinb", [128, T], BF16, isOutput=False)
    psw_d = nc.declare_dram_parameter("pswap", [128, 128], BF16, isOutput=False)
    dbg_d = nc.declare_dram_parameter("dbg", [128, 64], F32, isOutput=True)
    out_d = nc.declare_dram_parameter("out", [T, C], BF16, isOutput=True)

    with tile.TileContext(nc) as tc:
        with (
            tc.tile_pool(name="const", bufs=1) as const,
            tc.tile_pool(name="wpool", bufs=1) as wpool,
            tc.tile_pool(name="xtp", bufs=2) as xtp,
            tc.tile_pool(name="qkp", bufs=1) as qkp,
            tc.tile_pool(name="vap", bufs=1) as vap,
            tc.tile_pool(name="ytp", bufs=1) as ytp,
            tc.tile_pool(name="ptp", bufs=4) as ptp,
            tc.tile_pool(name="npool", bufs=2) as npool,
            tc.tile_pool(name="opool", bufs=3) as opool,
            tc.tile_pool(name="ps", bufs=2, space="PSUM") as ps,
        ):
            # ---- engine-generated constants (no DMA) ----
            tri_f = const.tile([128, 128], F32)
            make_upper_triangular(nc, tri_f, val=1.0, diag=True)  # 1 if i<=j
            tri = const.tile([128, 128], BF16)
            nc.vector.tensor_copy(tri[:], tri_f[:])
            wrm = const.tile([128, QTILE], BF16)
            nc.gpsimd.memset(wrm[:], 0.25)

            # ---- DMA loads, ordered by first use ----
            xt0 = xtp.tile([128, CB, QTILE], BF16, name="xt_sb", tag="xt")
            nc.sync.dma_start(
                out=xt0[:],
                in_=xt_d.ap()[0:128, :].rearrange("p (cb t) -> p cb t", cb=CB),
            )
            wq_b = wpool.tile([128, CB, HDB, 128], BF16)
            for hb in range(HDB):
                nc.sync.dma_start(
                    out=wq_b[:, :, hb, :],
                    in_=wq_d.ap()[hb * 128 : (hb + 1) * 128, :].rearrange(
                        "p (cb m) -> p cb m", cb=CB
                    ),
                )
            cos_t = const.tile([128, T], BF16)
            sin_t = const.tile([128, T], BF16)
            nc.sync.dma_start(out=cos_t[:], in_=cos_d[:])
            nc.sync.dma_start(out=sin_t[:], in_=sin_d[:])
            psw = const.tile([128, 128], BF16)
            nc.sync.dma_start(out=psw[:], in_=psw_d[:])
            wk_b = wpool.tile([128, CB, HDB, 128], BF16)
            for hb in range(HDB):
                nc.sync.dma_start(
                    out=wk_b[:, :, hb, :],
                    in_=wk_d.ap()[hb * 128 : (hb + 1) * 128, :].rearrange(
                        "p (cb m) -> p cb m", cb=CB
                    ),
                )
            wv_b = wpool.tile([128, CB, HD], BF16)
            nc.sync.dma_start(
                out=wv_b[:], in_=wv_d.ap().rearrange("p (cb m) -> p cb m", cb=CB)
            )
            wo_b = wpool.tile([128, HDB, C], BF16)
            nc.sync.dma_start(
                out=wo_b[:], in_=wo_d.ap().rearrange("p (hb c) -> p hb c", hb=HDB)
            )

            # ---- PE warmup: trip the HAM clock gate while DMAs stream.
            # Result goes to a dbg output so the chain is not dead code.
            wps = ps.tile([128, QTILE], F32, name="wps", tag="s1", bufs=2)
            for i in range(WARM_MM):
                nc.tensor.matmul(
                    wps[:], tri[:], wrm[:], start=(i == 0), stop=(i == WARM_MM - 1)
                )
            dbg_sb = const.tile([128, 64], F32)
            nc.vector.tensor_copy(dbg_sb[:], wps[:, 0:64])
            nc.sync.dma_start(out=dbg_d.ap()[:, :], in_=dbg_sb[:])

            # ---- persistent per-batch tensors ----
            qt = [qkp.tile([128, T], BF16, name=f"qt{hb}") for hb in range(HDB)]
            kt = [qkp.tile([128, T], BF16, name=f"kt{hb}") for hb in range(HDB)]
            yt = [ytp.tile([128, T], BF16, name=f"yt{hb}") for hb in range(HDB)]
            va = vap.tile([128, KB, NHL, D + 1], BF16)
            nc.gpsimd.memset(va[:, :, :, D : D + 1], 1.0)

            def s1_chunks(tt, cast_dve):
                """stage1 for one 512-token tile as (cost_ns, closure) chunks.

                Projects q/k (with fused rope) and v for tokens
                [tt*512, (tt+1)*512) from the host-transposed x tile."""
                t0 = tt * QTILE
                if tt == 0:
                    xt = xt0
                else:
                    xt = xtp.tile([128, CB, QTILE], BF16, name="xt_sb", tag="xt")

                chunks = []

                if tt != 0:
                    def dma_xt(xt=xt):
                        nc.sync.dma_start(
                            out=xt[:],
                            in_=xt_d.ap()[tt * 128 : (tt + 1) * 128, :].rearrange(
                                "p (cb t) -> p cb t", cb=CB
                            ),
                        )
                    chunks.append((50, dma_xt))

                # q/k projections with fused rope, per head-dim block. The
                # 8-MM accumulation is split into two 4-MM chunks so filler
                # granularity stays under ~1us of PE time.
                state = {}

                def proj_half(wr, hb, half, key):
                    if half == 0:
                        pj = ps.tile([128, QTILE], F32, name="pj", tag="s1", bufs=2)
                        state[key] = pj
                    else:
                        pj = state[key]
                    for cb in range(4 * half, 4 * half + 4):
                        nc.tensor.matmul(
                            pj[:], wr[:, cb, hb, :], xt[:, cb, :],
                            start=(cb == 0), stop=(cb == CB - 1),
                        )

                def rope(dst, hb, key):
                    pj = state.pop(key)
                    pjb = npool.tile([128, QTILE], BF16, name="pjb", tag="pjb", bufs=4)
                    if cast_dve:
                        nc.vector.tensor_copy(pjb[:], pj[:])
                    else:
                        nc.scalar.copy(pjb[:], pj[:])
                    pjs = npool.tile([128, QTILE], BF16, name="pjs", tag="pjs", bufs=2)
                    nc.vector.tensor_mul(pjs[:], pjb[:], sin_t[:, t0 : t0 + QTILE])
                    pjc = npool.tile([128, QTILE], BF16, name="pjc", tag="pjc", bufs=2)
                    nc.vector.tensor_mul(pjc[:], pjb[:], cos_t[:, t0 : t0 + QTILE])
                    rps = ps.tile([128, QTILE], F32, name="rps", tag="s1", bufs=2)
                    nc.tensor.matmul(rps[:], psw[:], pjs[:], start=True, stop=True)
                    nc.vector.tensor_add(dst[hb][:, t0 : t0 + QTILE], rps[:], pjc[:])

                # software-pipeline: rope(hb) is emitted after proj(hb+1)'s
                # first half so the PE is never waiting on the rope chain
                pend = None
                for wr, dst, nm in ((wq_b, qt, "q"), (wk_b, kt, "k")):
                    for hb in range(HDB):
                        key = (nm, hb)
                        chunks.append(
                            (900, lambda wr=wr, hb=hb, key=key: proj_half(wr, hb, 0, key))
                        )
                        if pend is not None:
                            chunks.append(pend)
                            pend = None
                        chunks.append(
                            (900, lambda wr=wr, hb=hb, key=key: proj_half(wr, hb, 1, key))
                        )
                        pend = (900, lambda dst=dst, hb=hb, key=key: rope(dst, hb, key))
                chunks.append(pend)

                # v projection straight into natural [t, hd] layout
                def vproj_half(tb, half, key):
                    if half == 0:
                        vj = ps.tile([128, HD], F32, name="vj", tag="s1", bufs=2)
                        state[key] = vj
                    else:
                        vj = state[key]
                    ts = slice(tb * 128, (tb + 1) * 128)
                    for cb in range(4 * half, 4 * half + 4):
                        nc.tensor.matmul(
                            vj[:], xt[:, cb, ts], wv_b[:, cb, :],
                            start=(cb == 0), stop=(cb == CB - 1),
                        )
                    if half == 1:
                        kv = tt * 4 + tb
                        vj = state.pop(key)
                        nc.vector.tensor_copy(
                            va[:, kv, :, 0:D],
                            vj[:].rearrange("p (h d) -> p h d", h=NHL),
                        )

                for tb in range(4):
                    key = ("v", tb)
                    chunks.append((900, lambda tb=tb, key=key: vproj_half(tb, 0, key)))
                    chunks.append((950, lambda tb=tb, key=key: vproj_half(tb, 1, key)))
                return chunks

            FQ = []  # filler queue of (cost_ns, closure)
            acc = [0.0]

            def fill(budget):
                acc[0] += budget
                while FQ and acc[0] >= FQ[0][0]:
                    cost, fn = FQ.pop(0)
                    fn()
                    acc[0] -= cost

            def flush():
                while FQ:
                    FQ.pop(0)[1]()
                acc[0] = 0.0

            def attention(j):
                """One 512-wide q tile of causal attention, all 8 heads,
                processed as 4 head-pairs packed into the two 64-row halves
                of the PE array."""
                q0 = j * QTILE
                nblk = 4 * (j + 1)
                rate = FILL_RATE[j]
                for hb in range(HDB):
                    hA, hB = 2 * hb, 2 * hb + 1
                    ytA = ps.tile([D + 1, QTILE], F32, name="ytA", tag="yt", bufs=2)
                    ytB = ps.tile([D + 1, QTILE], F32, name="ytB", tag="yt", bufs=2)
                    for k in range(nblk):
                        fill(rate)
                        m = k - 4 * j
                        e0 = 0 if m < 0 else m * 128
                        st = ps.tile(
                            [128, 2 * QTILE], F32, name="st", tag="st", bufs=2
                        )
                        pt = ptp.tile([128, 2 * QTILE], BF16, name="pt", bufs=4)
                        ks = slice(k * 128, (k + 1) * 128)
                        qs = slice(q0 + e0, q0 + QTILE)
                        # two heads concurrently in the two 64-row halves
                        nc.tensor.matmul(
                            st[:, e0:QTILE],
                            kt[hb][0:D, ks], qt[hb][0:D, qs],
                            start=True, stop=True,
                        )
                        nc.tensor.matmul(
                            st[:, QTILE + e0 : 2 * QTILE],
                            kt[hb][D:128, ks], qt[hb][D:128, qs],
                            start=True, stop=True,
                        )
                        if e0 == 0:
                            nc.scalar.activation(pt[:], st[:], EXP, scale=0.125)
                        else:
                            w = QTILE - e0
                            sv = st[:].rearrange("p (h w) -> p h w", h=2)[:, :, e0:]
                            pv = pt[:].rearrange("p (h w) -> p h w", h=2)[:, :, e0:]
                            nc.scalar.activation(pv, sv, EXP, scale=0.125)
                        if m >= 0:
                            for half in range(2):
                                o = half * QTILE + e0
                                nc.vector.tensor_mul(
                                    pt[:, o : o + 128], pt[:, o : o + 128], tri[:]
                                )
                        nc.tensor.matmul(
                            ytA[:, e0:QTILE],
                            va[:, k, hA, :], pt[:, e0:QTILE],
                            start=(k == 0), stop=(k == nblk - 1),
                        )
                        nc.tensor.matmul(
                            ytB[:, e0:QTILE],
                            va[:, k, hB, :], pt[:, QTILE + e0 : 2 * QTILE],
                            start=(k == 0), stop=(k == nblk - 1),
                        )
                    # softmax normalization: row sums sit in partition D of
                    # the PSUM tile; reciprocal straight off PSUM, partition
                    # broadcast on gpsimd, final scale on DVE
                    for yps, hp in ((ytA, 0), (ytB, D)):
                        rec = npool.tile([1, QTILE], F32, name="rec", tag="rec", bufs=4)
                        nc.vector.reciprocal(rec[:], yps[D : D + 1, :])
                        rbc = npool.tile([D, QTILE], F32, name="rbc", tag="rbc", bufs=4)
                        nc.gpsimd.partition_broadcast(rbc[:], rec[:])
                        nc.vector.tensor_mul(
                            yt[hb][hp : hp + D, q0 : q0 + QTILE], yps[0:D, :], rbc[:]
                        )

            def op_tiles(jo):
                """Output projection for q tile jo as filler chunks."""
                chunks = []

                def op_tile(tb, co):
                    ts = slice(tb * 128, (tb + 1) * 128)
                    op = ps.tile([128, QTILE], F32, name="op", tag="s1", bufs=2)
                    for hb in range(HDB):
                        nc.tensor.matmul(
                            op[:],
                            yt[hb][:, ts],
                            wo_b[:, hb, co * QTILE : (co + 1) * QTILE],
                            start=(hb == 0), stop=(hb == HDB - 1),
                        )
                    ot = opool.tile([128, QTILE], BF16, name="ot")
                    nc.vector.tensor_copy(ot[:], op[:])
                    nc.sync.dma_start(
                        out=out_d.ap()[ts, co * QTILE : (co + 1) * QTILE],
                        in_=ot[:],
                    )

                for tb in range(4 * jo, 4 * (jo + 1)):
                    for co in range(C // QTILE):
                        chunks.append((950, lambda tb=tb, co=co: op_tile(tb, co)))
                return chunks

            # ---- emission schedule ----
            # att(j) only needs kv blocks 0..4j+3 and q tile j, so att(0)
            # follows s1(0) immediately and later stage1/outproj tiles ride
            # inside the exp-bound attention loops as PE fillers.
            for _, fn in s1_chunks(0, cast_dve=False):
                fn()
            FQ.extend(s1_chunks(1, cast_dve=True))
            attention(0)
            flush()
            FQ.extend(s1_chunks(2, cast_dve=True))
            attention(1)
            flush()
            FQ.extend(s1_chunks(3, cast_dve=True))
            FQ.extend(op_tiles(0))
            attention(2)
            flush()
            FQ.extend(op_tiles(1))
            FQ.extend(op_tiles(2))
            attention(3)
            flush()
            for _, fn in op_tiles(3):
                fn()
    nc.finalize()
    return nc


def _rope_tables():
    freqs = 1.0 / (10000.0 ** (np.arange(0, D, 2, dtype=np.float64) / D))  # [32]
    grid = np.arange(T, dtype=np.float64)[:, None] * freqs[None, :]  # [T, 32]
    cos = np.cos(grid)
    sin = np.sin(grid)
    # row d uses freq d//2; sin sign: + for even d, - for odd d
    cos_b = np.repeat(cos.T, 2, axis=0)  # [64, T]
    sin_b = np.repeat(sin.T, 2, axis=0)
    sin_b[1::2] *= -1.0
    cos_hd = np.tile(cos_b, (2, 1)).astype(BF)  # [128, T]
    sin_hd = np.tile(sin_b, (2, 1)).astype(BF)
    return np.ascontiguousarray(cos_hd), np.ascontiguousarray(sin_hd)


def _pswap():
    p = np.zeros((128, 128), dtype=np.float32)
    idx = np.arange(0, 128, 2)
    p[idx, idx + 1] = 1.0
    p[idx + 1, idx] = 1.0
    return p.astype(BF)


def kernel(x, wq, wk, wv, wo):
    if "nc" not in _CACHE:
        _CACHE["nc"] = build()
    nc = _CACHE["nc"]

    cos_hd, sin_hd = _rope_tables()
    psw = _pswap()
    x_bf = np.ascontiguousarray(x, dtype=np.float32).astype(BF)
    core_ids = list(range(NCORES))
    in_maps = []
    for c in core_ids:
        b, hh = c // 2, c % 2
        sl = slice(hh * HD, (hh + 1) * HD)
        xT = x_bf[b].T  # [C, T]
        xt_host = np.ascontiguousarray(
            xT.reshape(CB, 128, NJ, QTILE).transpose(2, 1, 0, 3).reshape(
                NJ * 128, CB * QTILE
            )
        )
        wqt = wq[sl, :].T.astype(BF)  # [C, HD]
        wkt = wk[sl, :].T.astype(BF)
        wq_host = np.ascontiguousarray(
            wqt.reshape(CB, 128, HDB, 128).transpose(2, 1, 0, 3).reshape(
                HDB * 128, CB * 128
            )
        )
        wk_host = np.ascontiguousarray(
            wkt.reshape(CB, 128, HDB, 128).transpose(2, 1, 0, 3).reshape(
                HDB * 128, CB * 128
            )
        )
        wvt = wv[sl, :].T.astype(BF)  # [C, HD]
        wv_host = np.ascontiguousarray(
            wvt.reshape(CB, 128, HD).transpose(1, 0, 2).reshape(128, CB * HD)
        )
        wot = wo[:, sl].T.astype(BF)  # [HD, C]
        wo_host = np.ascontiguousarray(
            wot.reshape(HDB, 128, C).transpose(1, 0, 2).reshape(128, HDB * C)
        )
        in_maps.append(
            {
                "xt": xt_host,
                "wqt": wq_host,
                "wkt": wk_host,
                "wvt": wv_host,
                "wot": wo_host,
                "cosb": cos_hd,
                "sinb": sin_hd,
                "pswap": psw,
            }
        )

    def _run():
        res = run_bass_kernel_spmd(nc, in_maps, core_ids).results
        out = np.zeros((B, T, C), dtype=np.float32)
        for c in core_ids:
            out[c // 2] += res[c]["out"].astype(np.float32)
        return out

    try:
        out = _run()
    except Exception:
        # transient NRT/device hiccup: retry once
        out = _run()
    if not np.isfinite(out).all():
        # rare cold-start device artifact: rerun once
        out = _run()
    return out


# revision 3
# speedup vs baseline: 1.3800x; 1.3800x over previous
"""Causal self-attention (B=4, T=2048, C=1024, 16 heads, interleaved RoPE)
on 8 trn2 NeuronCores.

Sharding: 4x2 grid (batch x head-half). Core c owns batch c//2 and heads
(c%2)*8 .. (c%2)*8+7 (512 head channels). Each core projects Q/K/V for its
8 heads, runs causal attention, and produces a partial [T, C] output via
its wo slice; the host sums the two partials per batch.

v2 changes vs v1 (340us):
- x^T is pre-transposed on the host and DMA'd directly into the [c, t]
  layout stage1 needs: the 128 PE transposes + 32 ACT copies are gone.
- Score matmuls are packed two heads at a time into the two 64-row halves
  of the PE array (K=64 row tiling via base_partition 0/64): the two MMs
  run concurrently, halving score PE time.
- Softmax normalization drops the two SBUF->SBUF DMA spread hops:
  reciprocal runs directly on the PSUM row-sum row, then gpsimd
  partition-broadcast.
- A PE warmup burst at t=0 (result DMA'd to a dbg output so it is not
  dead code) trips the HAM clock gate at ~4us instead of ~21us.
- stage1/outproj work is emitted as fine-grained filler chunks inside the
  (ACT-exp-bound) attention k-loops so the PE never starves.
- Output tensor is bf16 (halves the output DMA).

Self-contained: hardcoded shapes, no reads of /root/problem/*.
"""
import numpy as np
import ml_dtypes

import concourse.bacc as bacc
import concourse.mybir as mybir
import concourse.tile as tile
from concourse.bass_utils import run_bass_kernel_spmd
from concourse.masks import make_upper_triangular

B, T, C = 4, 2048, 1024
NH, D = 16, 64
NCORES = 8
NHL = 8  # heads per core
HD = NHL * D  # per-core head channels = 512
HDB = HD // 128  # head-dim partition blocks = 4
QTILE = 512
KB = T // 128  # kv blocks = 16
NJ = T // QTILE  # q tiles = 4
CB = C // 128  # channel blocks = 8
F32 = mybir.dt.float32
BF16 = mybir.dt.bfloat16
EXP = mybir.ActivationFunctionType.Exp
BF = ml_dtypes.bfloat16

WARM_MM = 12  # PE warmup matmuls (N=512, cold ~426ns each)
# filler budget (ns of PE work) released per attention k-group, per phase
FILL_RATE = (1500, 800, 700, 260)

_CACHE = {}


def build():
    nc = bacc.Bacc(None, target_bir_lowering=False)
    xt_d = nc.declare_dram_parameter("xt", [NJ * 128, CB * QTILE], BF16, isOutput=False)
    wq_d = nc.declare_dram_parameter("wqt", [HDB * 128, CB * 128], BF16, isOutput=False)
    wk_d = nc.declare_dram_parameter("wkt", [HDB * 128, CB * 128], BF16, isOutput=False)
    wv_d = nc.declare_dram_parameter("wvt", [128, CB * HD], BF16, isOutput=False)
    wo_d = nc.declare_dram_parameter("wot", [128, HDB * C], BF16, isOutput=False)
    cos_d = nc.declare_dram_parameter("cosb", [128, T], BF16, isOutput=False)
    sin_d = nc.declare_dram_parameter("sinb", [128, T], BF16, isOutput=False)
    psw_d = nc.declare_dram_parameter("pswap", [128, 128], BF16, isOutput=False)
    dbg_d = nc.declare_dram_parameter("dbg", [128, 64], F32, isOutput=True)
    out_d = nc.declare_dram_parameter("out", [T, C], BF16, isOutput=True)

    with tile.TileContext(nc) as tc:
        with (
            tc.tile_pool(name="const", bufs=1) as const,
            tc.tile_pool(name="wpool", bufs=1) as wpool,
            tc.tile_pool(name="xtp", bufs=2) as xtp,
            tc.tile_pool(name="qkp", bufs=1) as qkp,
            tc.tile_pool(name="vap", bufs=1) as vap,
            tc.tile_pool(name="ytp", bufs=1) as ytp,
            tc.tile_pool(name="ptp", bufs=4) as ptp,
            tc.tile_pool(name="npool", bufs=2) as npool,
            tc.tile_pool(name="opool", bufs=3) as opool,
            tc.tile_pool(name="ps", bufs=2, space="PSUM") as ps,
        ):
            # ---- engine-generated constants (no DMA) ----
            tri_f = const.tile([128, 128], F32)
            make_upper_triangular(nc, tri_f, val=1.0, diag=True)  # 1 if i<=j
            tri = const.tile([128, 128], BF16)
            nc.vector.tensor_copy(tri[:], tri_f[:])
            wrm = const.tile([128, QTILE], BF16)
            nc.gpsimd.memset(wrm[:], 0.25)

            # ---- DMA loads, ordered by first use ----
            xt0 = xtp.tile([128, CB, QTILE], BF16, name="xt_sb", tag="xt")
            nc.sync.dma_start(
                out=xt0[:],
                in_=xt_d.ap()[0:128, :].rearrange("p (cb t) -> p cb t", cb=CB),
            )
            wq_b = wpool.tile([128, CB, HDB, 128], BF16)
            for hb in range(HDB):
                nc.sync.dma_start(
                    out=wq_b[:, :, hb, :],
                    in_=wq_d.ap()[hb * 128 : (hb + 1) * 128, :].rearrange(
                        "p (cb m) -> p cb m", cb=CB
                    ),
                )
            cos_t = const.tile([128, T], BF16)
            sin_t = const.tile([128, T], BF16)
            nc.sync.dma_start(out=cos_t[:], in_=cos_d[:])
            nc.sync.dma_start(out=sin_t[:], in_=sin_d[:])
            psw = const.tile([128, 128], BF16)
            nc.sync.dma_start(out=psw[:], in_=psw_d[:])
            wk_b = wpool.tile([128, CB, HDB, 128], BF16)
            for hb in range(HDB):
                nc.sync.dma_start(
                    out=wk_b[:, :, hb, :],
                    in_=wk_d.ap()[hb * 128 : (hb + 1) * 128, :].rearrange(
                        "p (cb m) -> p cb m", cb=CB
                    ),
                )
            wv_b = wpool.tile([128, CB, HD], BF16)
            nc.sync.dma_start(
                out=wv_b[:], in_=wv_d.ap().rearrange("p (cb m) -> p cb m", cb=CB)
            )
            wo_b = wpool.tile([128, HDB, C], BF16)
            nc.sync.dma_start(
                out=wo_b[:], in_=wo_d.ap().rearrange("p (hb c) -> p hb c", hb=HDB)
            )

            # ---- PE warmup: trip the HAM clock gate while DMAs stream.
            # Result goes to a dbg output so the chain is not dead code.
            wps = ps.tile([128, QTILE], F32, name="wps", tag="s1", bufs=2)
            for i in range(WARM_MM):
                nc.tensor.matmul(
                    wps[:], tri[:], wrm[:], start=(i == 0), stop=(i == WARM_MM - 1)
                )
            dbg_sb = const.tile([128, 64], F32)
            nc.vector.tensor_copy(dbg_sb[:], wps[:, 0:64])
            nc.sync.dma_start(out=dbg_d.ap()[:, :], in_=dbg_sb[:])

            # ---- persistent per-batch tensors ----
            qt = [qkp.tile([128, T], BF16, name=f"qt{hb}") for hb in range(HDB)]
            kt = [qkp.tile([128, T], BF16, name=f"kt{hb}") for hb in range(HDB)]
            yt = [ytp.tile([128, T], BF16, name=f"yt{hb}") for hb in range(HDB)]
            va = vap.tile([128, KB, NHL, D + 1], BF16)
            nc.gpsimd.memset(va[:, :, :, D : D + 1], 1.0)

            def s1_chunks(tt, cast_dve):
                """stage1 for one 512-token tile as (cost_ns, closure) chunks.

                Projects q/k (with fused rope) and v for tokens
                [tt*512, (tt+1)*512) from the host-transposed x tile."""
                t0 = tt * QTILE
                if tt == 0:
                    xt = xt0
                else:
                    xt = xtp.tile([128, CB, QTILE], BF16, name="xt_sb", tag="xt")

                chunks = []

                if tt != 0:
                    def dma_xt(xt=xt):
                        nc.sync.dma_start(
                            out=xt[:],
                            in_=xt_d.ap()[tt * 128 : (tt + 1) * 128, :].rearrange(
                                "p (cb t) -> p cb t", cb=CB
                            ),
                        )
                    chunks.append((50, dma_xt))

                # q/k projections with fused rope, per head-dim block. The
                # 8-MM accumulation is split into two 4-MM chunks so filler
                # granularity stays under ~1us of PE time.
                state = {}

                def proj_half(wr, hb, half, key):
                    if half == 0:
                        pj = ps.tile([128, QTILE], F32, name="pj", tag="s1", bufs=2)
                        state[key] = pj
                    else:
                        pj = state[key]
                    for cb in range(4 * half, 4 * half + 4):
                        nc.tensor.matmul(
                            pj[:], wr[:, cb, hb, :], xt[:, cb, :],
                            start=(cb == 0), stop=(cb == CB - 1),
                        )

                def rope(dst, hb, key):
                    pj = state.pop(key)
                    pjb = npool.tile([128, QTILE], BF16, name="pjb", tag="pjb", bufs=4)
                    if cast_dve:
                        nc.vector.tensor_copy(pjb[:], pj[:])
                    else:
                        nc.scalar.copy(pjb[:], pj[:])
                    pjs = npool.tile([128, QTILE], BF16, name="pjs", tag="pjs", bufs=2)
                    nc.vector.tensor_mul(pjs[:], pjb[:], sin_t[:, t0 : t0 + QTILE])
                    pjc = npool.tile([128, QTILE], BF16, name="pjc", tag="pjc", bufs=2)
                    nc.vector.tensor_mul(pjc[:], pjb[:], cos_t[:, t0 : t0 + QTILE])
                    rps = ps.tile([128, QTILE], F32, name="rps", tag="s1", bufs=2)
                    nc.tensor.matmul(rps[:], psw[:], pjs[:], start=True, stop=True)
                    nc.vector.tensor_add(dst[hb][:, t0 : t0 + QTILE], rps[:], pjc[:])

                # software-pipeline: rope(hb) is emitted after proj(hb+1)'s
                # first half so the PE is never waiting on the rope chain
                pend = None
                for wr, dst, nm in ((wq_b, qt, "q"), (wk_b, kt, "k")):
                    for hb in range(HDB):
                        key = (nm, hb)
                        chunks.append(
                            (900, lambda wr=wr, hb=hb, key=key: proj_half(wr, hb, 0, key))
                        )
                        if pend is not None:
                            chunks.append(pend)
                            pend = None
                        chunks.append(
                            (900, lambda wr=wr, hb=hb, key=key: proj_half(wr, hb, 1, key))
                        )
                        pend = (900, lambda dst=dst, hb=hb, key=key: rope(dst, hb, key))
                chunks.append(pend)

                # v projection straight into natural [t, hd] layout
                def vproj_half(tb, half, key):
                    if half == 0:
                        vj = ps.tile([128, HD], F32, name="vj", tag="s1", bufs=2)
                        state[key] = vj
                    else:
                        vj = state[key]
                    ts = slice(tb * 128, (tb + 1) * 128)
                    for cb in range(4 * half, 4 * half + 4):
                        nc.tensor.matmul(
                            vj[:], xt[:, cb, ts], wv_b[:, cb, :],
                            start=(cb == 0), stop=(cb == CB - 1),
                        )
                    if half == 1:
                        kv = tt * 4 + tb
                        vj = state.pop(key)
                        nc.vector.tensor_copy(
                            va[:, kv, :, 0:D],
                            vj[:].rearrange("p (h d) -> p h d", h=NHL),
                        )

                for tb in range(4):
                    key = ("v", tb)
                    chunks.append((900, lambda tb=tb, key=key: vproj_half(tb, 0, key)))
                    chunks.append((950, lambda tb=tb, key=key: vproj_half(tb, 1, key)))
                return chunks

            FQ = []  # filler queue of (cost_ns, closure)
            acc = [0.0]

            def fill(budget):
                acc[0] += budget
                while FQ and acc[0] >= FQ[0][0]:
                    cost, fn = FQ.pop(0)
                    fn()
                    acc[0] -= cost

            def flush():
                while FQ:
                    FQ.pop(0)[1]()
                acc[0] = 0.0

            def attention(j):
                """One 512-wide q tile of causal attention, all 8 heads,
                processed as 4 head-pairs packed into the two 64-row halves
                of the PE array."""
                q0 = j * QTILE
                nblk = 4 * (j + 1)
                rate = FILL_RATE[j]
                for hb in range(HDB):
                    hA, hB = 2 * hb, 2 * hb + 1
                    ytA = ps.tile([D + 1, QTILE], F32, name="ytA", tag="yt", bufs=2)
                    ytB = ps.tile([D + 1, QTILE], F32, name="ytB", tag="yt", bufs=2)
                    for k in range(nblk):
                        fill(rate)
                        m = k - 4 * j
                        e0 = 0 if m < 0 else m * 128
                        st = ps.tile(
                            [128, 2 * QTILE], F32, name="st", tag="st", bufs=2
                        )
                        pt = ptp.tile([128, 2 * QTILE], BF16, name="pt", bufs=4)
                        ks = slice(k * 128, (k + 1) * 128)
                        qs = slice(q0 + e0, q0 + QTILE)
                        # two heads concurrently in the two 64-row halves
                        nc.tensor.matmul(
                            st[:, e0:QTILE],
                            kt[hb][0:D, ks], qt[hb][0:D, qs],
                            start=True, stop=True,
                        )
                        nc.tensor.matmul(
                            st[:, QTILE + e0 : 2 * QTILE],
                            kt[hb][D:128, ks], qt[hb][D:128, qs],
                            start=True, stop=True,
                        )
                        if e0 == 0:
                            nc.scalar.activation(pt[:], st[:], EXP, scale=0.125)
                        else:
                            w = QTILE - e0
                            sv = st[:].rearrange("p (h w) -> p h w", h=2)[:, :, e0:]
                            pv = pt[:].rearrange("p (h w) -> p h w", h=2)[:, :, e0:]
                            nc.scalar.activation(pv, sv, EXP, scale=0.125)
                        if m >= 0:
                            for half in range(2):
                                o = half * QTILE + e0
                                nc.vector.tensor_mul(
                                    pt[:, o : o + 128], pt[:, o : o + 128], tri[:]
                                )
                        nc.tensor.matmul(
                            ytA[:, e0:QTILE],
                            va[:, k, hA, :], pt[:, e0:QTILE],
                            start=(k == 0), stop=(k == nblk - 1),
                        )
                        nc.tensor.matmul(
                            ytB[:, e0:QTILE],
                            va[:, k, hB, :], pt[:, QTILE + e0 : 2 * QTILE],
                            start=(k == 0), stop=(k == nblk - 1),
                        )
                    # softmax normalization: row sums sit in partition D of
                    # the PSUM tile. DVE reciprocal throughput goes with the
                    # FREE-dim size (~6.5ns/elem), so the sums are DMA-spread
                    # to [128, 4] first (176ns recip), gathered back to a row
                    # and partition-broadcast on gpsimd. The yu copy frees
                    # the PSUM accumulator before the long-latency chain.
                    for yps, hp in ((ytA, 0), (ytB, D)):
                        yu = npool.tile([D + 1, QTILE], F32, name="yu", tag="yu", bufs=4)
                        nc.vector.tensor_copy(yu[:], yps[:])
                        s128 = npool.tile([128, 4], F32, name="s128", tag="s128", bufs=4)
                        nc.sync.dma_start(out=s128[:], in_=yu[D : D + 1, :])
                        r128 = npool.tile([128, 4], F32, name="r128", tag="r128", bufs=4)
                        nc.vector.reciprocal(r128[:], s128[:])
                        rrow = npool.tile([1, QTILE], F32, name="rrow", tag="rrow", bufs=4)
                        nc.sync.dma_start(out=rrow[:], in_=r128[:])
                        rbc = npool.tile([D, QTILE], F32, name="rbc", tag="rbc", bufs=4)
                        nc.gpsimd.partition_broadcast(rbc[:], rrow[:])
                        nc.vector.tensor_mul(
                            yt[hb][hp : hp + D, q0 : q0 + QTILE], yu[0:D, :], rbc[:]
                        )

            def op_tiles(jo):
                """Output projection for q tile jo as filler chunks."""
                chunks = []

                def op_tile(tb, co):
                    ts = slice(tb * 128, (tb + 1) * 128)
                    op = ps.tile([128, QTILE], F32, name="op", tag="s1", bufs=2)
                    for hb in range(HDB):
                        nc.tensor.matmul(
                            op[:],
                            yt[hb][:, ts],
                            wo_b[:, hb, co * QTILE : (co + 1) * QTILE],
                            start=(hb == 0), stop=(hb == HDB - 1),
                        )
                    ot = opool.tile([128, QTILE], BF16, name="ot")
                    nc.vector.tensor_copy(ot[:], op[:])
                    nc.sync.dma_start(
                        out=out_d.ap()[ts, co * QTILE : (co + 1) * QTILE],
                        in_=ot[:],
                    )

                for tb in range(4 * jo, 4 * (jo + 1)):
                    for co in range(C // QTILE):
                        chunks.append((950, lambda tb=tb, co=co: op_tile(tb, co)))
                return chunks

            # ---- emission schedule ----
            # att(j) only needs kv blocks 0..4j+3 and q tile j, so att(0)
            # follows s1(0) immediately and later stage1/outproj tiles ride
            # inside the exp-bound attention loops as PE fillers.
            for _, fn in s1_chunks(0, cast_dve=False):
                fn()
            FQ.extend(s1_chunks(1, cast_dve=True))
            attention(0)
            flush()
            FQ.extend(s1_chunks(2, cast_dve=True))
            attention(1)
            flush()
            FQ.extend(s1_chunks(3, cast_dve=True))
            FQ.extend(op_tiles(0))
            attention(2)
            flush()
            FQ.extend(op_tiles(1))
            FQ.extend(op_tiles(2))
            attention(3)
            flush()
            for _, fn in op_tiles(3):
                fn()
    nc.finalize()
    return nc


def _rope_tables():
    freqs = 1.0 / (10000.0 ** (np.arange(0, D, 2, dtype=np.float64) / D))  # [32]
    grid = np.arange(T, dtype=np.float64)[:, None] * freqs[None, :]  # [T, 32]
    cos = np.cos(grid)
    sin = np.sin(grid)
    # row d uses freq d//2; sin sign: + for even d, - for odd d
    cos_b = np.repeat(cos.T, 2, axis=0)  # [64, T]
    sin_b = np.repeat(sin.T, 2, axis=0)
    sin_b[1::2] *= -1.0
    cos_hd = np.tile(cos_b, (2, 1)).astype(BF)  # [128, T]
    sin_hd = np.tile(sin_b, (2, 1)).astype(BF)
    return np.ascontiguousarray(cos_hd), np.ascontiguousarray(sin_hd)


def _pswap():
    p = np.zeros((128, 128), dtype=np.float32)
    idx = np.arange(0, 128, 2)
    p[idx, idx + 1] = 1.0
    p[idx + 1, idx] = 1.0
    return p.astype(BF)


def kernel(x, wq, wk, wv, wo):
    if "nc" not in _CACHE:
        _CACHE["nc"] = build()
    nc = _CACHE["nc"]

    cos_hd, sin_hd = _rope_tables()
    psw = _pswap()
    x_bf = np.ascontiguousarray(x, dtype=np.float32).astype(BF)
    core_ids = list(range(NCORES))
    in_maps = []
    for c in core_ids:
        b, hh = c // 2, c % 2
        sl = slice(hh * HD, (hh + 1) * HD)
        xT = x_bf[b].T  # [C, T]
        xt_host = np.ascontiguousarray(
            xT.reshape(CB, 128, NJ, QTILE).transpose(2, 1, 0, 3).reshape(
                NJ * 128, CB * QTILE
            )
        )
        wqt = wq[sl, :].T.astype(BF)  # [C, HD]
        wkt = wk[sl, :].T.astype(BF)
        wq_host = np.ascontiguousarray(
            wqt.reshape(CB, 128, HDB, 128).transpose(2, 1, 0, 3).reshape(
                HDB * 128, CB * 128
            )
        )
        wk_host = np.ascontiguousarray(
            wkt.reshape(CB, 128, HDB, 128).transpose(2, 1, 0, 3).reshape(
                HDB * 128, CB * 128
            )
        )
        wvt = wv[sl, :].T.astype(BF)  # [C, HD]
        wv_host = np.ascontiguousarray(
            wvt.reshape(CB, 128, HD).transpose(1, 0, 2).reshape(128, CB * HD)
        )
        wot = wo[:, sl].T.astype(BF)  # [HD, C]
        wo_host = np.ascontiguousarray(
            wot.reshape(HDB, 128, C).transpose(1, 0, 2).reshape(128, HDB * C)
        )
        in_maps.append(
            {
                "xt": xt_host,
                "wqt": wq_host,
                "wkt": wk_host,
                "wvt": wv_host,
                "wot": wo_host,
                "cosb": cos_hd,
                "sinb": sin_hd,
                "pswap": psw,
            }
        )

    def _run():
        res = run_bass_kernel_spmd(nc, in_maps, core_ids).results
        out = np.zeros((B, T, C), dtype=np.float32)
        for c in core_ids:
            out[c // 2] += res[c]["out"].astype(np.float32)
        return out

    try:
        out = _run()
    except Exception:
        # transient NRT/device hiccup: retry once
        out = _run()
    if not np.isfinite(out).all():
        # rare cold-start device artifact: rerun once
        out = _run()
    return out
